# revision 18
# baseline (speedup 1.0000x reference)
"""GNN message-passing aggregator on 8 Trainium2 NeuronCores.

  h = relu(relu(z @ U1 + c1) @ U2 + c2)
  z = segment_sum(relu(relu(y[src] @ W1 + b1) @ W2 + b2), dst)

Strategy (v2):
  * MLP(y[src]) == MLP(y)[src]: compute the pre-MLP once per node (m table),
    then per-edge work collapses to gather m[src] + segment-sum by dst.
  * Edges sharded by dst ownership (6250 nodes/core): segment-sum is
    core-local, no collectives.
  * The m table is fp16 stored as PAIR rows [25024, 128] (node 2r | node
    2r+1, 256 B/row): pair index = src>>1 fits int16, and 256 B satisfies the
    dma_gather elem-size constraint.  Columns are grouped per (dst-window,
    src-parity) so each column's matmul reads the correct 64-elem half.
  * dma_gather descriptor generation runs on the GPSIMD Q7 core pair
    {2q, 2q+1} selected by queue_num.  With num_swdge_queues=4 and ops
    round-robined over queues 0-3 all 8 Q7 cores generate concurrently:
    measured 2.6 ns/idx vs 8.8 ns/idx single-queue.  This is the kernel's
    critical resource (~157k indices/core).
  * Gather ops never span groups; pad slots use idx=-1 which the ucode trims
    at the op tail, so padding costs no descriptor time.  Per-op dependency
    on the m-table chunk covering its max row lets gathers start while
    phase 1 is still writing the table.
  * All matmuls fp16 (fp32 PSUM): phase 1, the one-hot P scatter matmuls
    (built on DVE from host-streamed nodeid vs iota), and the update MLP.
    Phase 3 is interleaved into the window loop so it pipelines into the
    gather stream; final h is fp32.
"""

import os

import numpy as np

# ---------------------------------------------------------------- constants
N_NODES = 50000
D = 64
NC = 8                      # cores
W = 128                     # window size == psum partitions
OP_COLS = 8                 # gather columns per dma_gather op (8*128=1024 idx)
CHUNK = 512                 # dense-MLP T-major matmul chunk (nodes)
GRP = 4                     # windows per phase-3 group
NQ = 4                      # SWDGE queues (ucode max; 4 Q7 core pairs)
GBUFS = 12                  # gather pool depth
WBLK = 16                   # windows per PSUM accumulator block

_COMPILED = {}


def _dims():
    npc = N_NODES // NC
    nw = (npc + W - 1) // W
    npad = nw * W
    mt_rows = ((N_NODES + 127) // 128) * 128
    return npc, nw, npad, mt_rows


# ------------------------------------------------------------ host schedule
def _host_schedule(src, dst):
    """Shard edges by dst ownership, balance nodes into windows, pack columns
    per (window, src-parity) group, cut ops, compute per-op chunk deps.

    Returns (sched, per_core): sched is the shared compile-time schedule
    (identical across cores); per_core holds the input arrays per core.
    """
    NPC, NW, NPAD, MT_ROWS = _dims()
    MT2 = MT_ROWS // 2

    percore_groups = []
    percore_perm = []
    cnt = np.zeros((NC, NW, 2), np.int64)

    for c in range(NC):
        lo_n, hi_n = c * NPC, (c + 1) * NPC
        sel = (dst >= lo_n) & (dst < hi_n)
        s = src[sel].astype(np.int64)
        d = (dst[sel] - lo_n).astype(np.int64)
        deg = np.bincount(d, minlength=NPC)

        # balance nodes into NW windows by degree (greedy, descending)
        order = np.argsort(-deg, kind="stable")
        wload = np.zeros(NW, np.int64)
        wcount = np.zeros(NW, np.int64)
        assign = np.zeros(NPC, np.int64)
        label = np.zeros(NPC, np.int64)
        for n in order:
            wavail = np.flatnonzero(wcount < W)
            wsel = wavail[np.argmin(wload[wavail])]
            assign[n] = wsel
            label[n] = wcount[wsel]
            wcount[wsel] += 1
            wload[wsel] += deg[n]

        ew = assign[d]
        par = s & 1
        groups = {}
        for wdw in range(NW):
            m_w = ew == wdw
            for t in range(2):
                m_t = m_w & (par == t)
                es = (s[m_t] >> 1)                  # pair-row index
                el = label[d[m_t]]
                o = np.argsort(es, kind="stable")   # src-sorted for locality
                groups[(wdw, t)] = (es[o], el[o])
                cnt[c, wdw, t] = (len(es) + W - 1) // W
        percore_groups.append(groups)
        perm = np.full(NPAD, -1, np.int64)
        perm[assign * W + label] = np.arange(NPC) + lo_n
        percore_perm.append(perm)

    C = cnt.max(axis=0)                             # [NW, 2] columns per group
    ncols = int(C.sum())

    # global column order: window BLOCKS of WBLK, layer-major inside a block
    # (layer l = l-th column of each group, ascending src rows) so early
    # gathers only need early m-table chunks
    cols = []                                       # (window, parity, j)
    blocks = []                                     # (w_lo, w_hi, col_lo, col_hi)
    for b0 in range(0, NW, WBLK):
        b1 = min(b0 + WBLK, NW)
        lo = len(cols)
        maxc = int(C[b0:b1].max())
        for j in range(maxc):
            for wdw in range(b0, b1):
                for t in range(2):
                    if j < int(C[wdw, t]):
                        cols.append((wdw, t, j))
        blocks.append((b0, b1, lo, len(cols)))
    assert len(cols) == ncols

    # ops are plain 8-column cuts of the global column list
    nops = (ncols + OP_COLS - 1) // OP_COLS

    # per-op chunk dependency: max pair-row gathered by any core in the op
    # (chunk ch writes pair rows [ch*CHUNK//2, (ch+1)*CHUNK//2))
    op_dep = []
    for o in range(nops):
        mx = 0
        for (wdw, t, j) in cols[o * OP_COLS:(o + 1) * OP_COLS]:
            for c in range(NC):
                es, _ = percore_groups[c][(wdw, t)]
                end = min((j + 1) * W, len(es))
                if end > j * W:
                    mx = max(mx, int(es[j * W:end].max()))
        op_dep.append(min(mx * 2 // CHUNK, (MT_ROWS - 1) // CHUNK))

    per_core = []
    for c in range(NC):
        groups = percore_groups[c]
        idx = np.zeros(ncols * W, np.int16)         # pad rows gather row 0
        nid = np.full(ncols * W, -1.0, np.float16)  # pad slots contribute 0
        for ci, (wdw, t, j) in enumerate(cols):
            es, el = groups[(wdw, t)]
            lo = j * W
            hi = min((j + 1) * W, len(es))
            n = hi - lo
            if n > 0:
                idx[ci * W:ci * W + n] = es[lo:hi].astype(np.int16)
                nid[ci * W:ci * W + n] = el[lo:hi].astype(np.float16)
        # idx wrapped per-op into [16, ni/16] replicated to 128 partitions
        wraps = []
        for o in range(nops):
            op = idx[o * OP_COLS * W:(o + 1) * OP_COLS * W]
            blk = op.reshape(len(op) // 16, 16).T        # [16, ni/16]
            wraps.append(np.tile(blk, (8, 1)))           # [128, ni/16]
        idx_2d = np.concatenate(wraps, axis=1)
        nodeid_2d = nid.reshape(ncols, W).T
        per_core.append({
            "idx": np.ascontiguousarray(idx_2d),
            "nodeid": np.ascontiguousarray(nodeid_2d),
            "perm": percore_perm[c],
        })

    sched = {"C": tuple(tuple(int(x) for x in row) for row in C),
             "cols": tuple(cols), "blocks": tuple(blocks),
             "op_dep": tuple(op_dep), "ncols": ncols}
    return sched, per_core


# ------------------------------------------------------------- bass program
def _build_program(sched):
    import concourse.bacc as bacc
    import concourse.mybir as mybir
    import concourse.tile as tile
    from concourse.masks import make_identity
    from concourse.tile import add_dep_helper

    f32 = mybir.dt.float32
    f16 = mybir.dt.float16
    i16 = mybir.dt.int16
    Relu = mybir.ActivationFunctionType.Relu

    NPC, NW, NPAD, MT_ROWS = _dims()
    MT2 = MT_ROWS // 2
    C = sched["C"]
    cols = sched["cols"]
    blocks = sched["blocks"]
    op_dep = sched["op_dep"]
    ncols = sched["ncols"]
    NCH = (MT_ROWS + CHUNK - 1) // CHUNK
    NGRP = (NW + GRP - 1) // GRP

    nc = bacc.Bacc(num_swdge_queues=NQ)
    yT_in = nc.dram_tensor("yT", [D + 1, MT_ROWS], f16, kind="ExternalInput")
    wb1_in = nc.dram_tensor("wb1", [D + 1, D], f16, kind="ExternalInput")
    wb2_in = nc.dram_tensor("wb2", [D + 1, D], f16, kind="ExternalInput")
    ub1_in = nc.dram_tensor("ub1", [D + 1, D], f16, kind="ExternalInput")
    ub2_in = nc.dram_tensor("ub2", [D + 1, D], f16, kind="ExternalInput")
    idx_in = nc.dram_tensor("idx", [128, ncols * 8], i16, kind="ExternalInput")
    nodeid_in = nc.dram_tensor("nodeid", [128, ncols], f16, kind="ExternalInput")
    iota_in = nc.dram_tensor("iota128", [128, 128], f16, kind="ExternalInput")
    m_dram = nc.dram_tensor("m_scratch", [MT2, 128], f16, kind="Internal")
    h_out = nc.dram_tensor("h_out", [NPAD, D], f32, kind="ExternalOutput")

    with tile.TileContext(nc) as tc:
        with tc.tile_pool(name="const", bufs=1) as cpool, \
             tc.tile_pool(name="idxp", bufs=1) as idxp, \
             tc.tile_pool(name="psb", bufs=2, space="PSUM") as psb_pool, \
             tc.tile_pool(name="pss", bufs=2, space="PSUM") as pss_pool, \
             tc.tile_pool(name="wps", bufs=1, space="PSUM") as wps, \
             tc.tile_pool(name="pstp", bufs=1, space="PSUM") as pstp:
            wb1 = cpool.tile([D + 1, D], f16, tag="wb1")
            wb2 = cpool.tile([D + 1, D], f16, tag="wb2")
            ub1 = cpool.tile([D + 1, D], f16, tag="ub1")
            ub2 = cpool.tile([D + 1, D], f16, tag="ub2")
            iota = cpool.tile([128, 128], f16, tag="iota")
            ident = cpool.tile([128, 128], f16, tag="ident")
            nc.sync.dma_start(out=wb1[:], in_=wb1_in[:])
            nc.sync.dma_start(out=wb2[:], in_=wb2_in[:])
            nc.sync.dma_start(out=ub1[:], in_=ub1_in[:])
            nc.sync.dma_start(out=ub2[:], in_=ub2_in[:])
            nc.sync.dma_start(out=iota[:], in_=iota_in[:])
            make_identity(nc, ident[:])

            idx_t = idxp.tile([128, ncols * 8], i16, tag="idx")
            nc.sync.dma_start(out=idx_t[:], in_=idx_in[:])
            nodeid_t = idxp.tile([128, ncols], f16, tag="nid")
            nc.sync.dma_start(out=nodeid_t[:], in_=nodeid_in[:])

            # ------------ phase 1: m = relu(relu(y@W1+b1)@W2+b2) -> m_dram ---
            m_writes = []
            with tc.tile_pool(name="p1y", bufs=3) as p1y, \
                 tc.tile_pool(name="p1h", bufs=1) as p1h, \
                 tc.tile_pool(name="p1m", bufs=3) as p1m:
                h1a = p1h.tile([D + 1, CHUNK], f16, tag="h1a")
                h1b = p1h.tile([D + 1, CHUNK], f16, tag="h1b")
                nc.gpsimd.memset(h1a[D:D + 1, :], 1.0)
                nc.gpsimd.memset(h1b[D:D + 1, :], 1.0)
                h1bufs = [h1a, h1b]
                mch = None
                pair_c0, pair_cw = 0, 0
                for ch in range(NCH):
                    c0 = ch * CHUNK
                    cw = min(CHUNK, MT_ROWS - c0)
                    ytile = p1y.tile([D + 1, CHUNK], f16, tag="ytile")
                    nc.sync.dma_start(out=ytile[:, :cw], in_=yT_in[:, c0:c0 + cw])
                    ps = psb_pool.tile([D, CHUNK], f32, tag="psb")
                    nc.tensor.matmul(out=ps[:, :cw], lhsT=wb1[:], rhs=ytile[:, :cw],
                                     start=True, stop=True)
                    h1c = h1bufs[ch % 2]
                    nc.scalar.activation(out=h1c[:D, :cw], in_=ps[:, :cw], func=Relu)
                    if ch % 2 == 0:
                        mch = p1m.tile([128, 2 * (CHUNK // 128) * D], f16,
                                       tag="mch")
                        pair_c0, pair_cw = c0, 0
                    nb = cw // 128
                    half = (ch % 2) * (CHUNK // 128) * D
                    ps2 = pss_pool.tile([128, (CHUNK // 128) * D], f32, tag="pss")
                    for i in range(nb):
                        nc.tensor.matmul(out=ps2[:, i * D:(i + 1) * D],
                                         lhsT=h1c[:, i * 128:(i + 1) * 128],
                                         rhs=wb2[:], start=True, stop=True)
                    nc.vector.tensor_scalar_max(
                        out=mch[:, half:half + nb * D], in0=ps2[:, :nb * D],
                        scalar1=0.0)
                    pair_cw += cw
                    if ch % 2 == 1 or ch == NCH - 1:
                        # write the fp16 pair-row table: node (t*128+p) ->
                        # pair row t*64+p//2, half p%2
                        wri = nc.scalar.dma_start(
                            out=m_dram[pair_c0 // 2:(pair_c0 + pair_cw) // 2, :]
                                .rearrange("(t ph) (pl d) -> (ph pl) t d",
                                           ph=64, pl=2),
                            in_=mch[:, :(pair_cw // 128) * D].rearrange(
                                "p (t d) -> p t d", d=D))
                        m_writes.append(wri)

            # gathers dep directly on the pair-write covering their max row:
            # writes share one HWDGE queue and each spans all 16 DMA engines,
            # so write K completing implies writes 0..K-1 completed
            junction = [m_writes[min(ch // 2, len(m_writes) - 1)]
                        for ch in range(NCH)]

            # ------------ phase 2 + 3 interleaved ---------------------------
            with tc.tile_pool(name="gpool", bufs=GBUFS) as gpool, \
                 tc.tile_pool(name="ppool", bufs=6) as ppool, \
                 tc.tile_pool(name="zg", bufs=3) as zgp, \
                 tc.tile_pool(name="p3z", bufs=1) as p3z, \
                 tc.tile_pool(name="p3g", bufs=1) as p3g, \
                 tc.tile_pool(name="p3h", bufs=2) as p3h:

                # phase-3 staging buffers with the ones-row preset ONCE, so
                # no per-group memset lands in the Pool queue between gathers
                zTgs = [p3z.tile([D + 1, GRP * 128], f16, tag=f"zTg{i}",
                                 name=f"zTg{i}") for i in range(2)]
                g1Tgs = [p3g.tile([D + 1, GRP * 128], f16, tag=f"g1Tg{i}",
                                  name=f"g1Tg{i}") for i in range(2)]
                for tl in zTgs + g1Tgs:
                    nc.vector.memset(tl[D:D + 1, :], 1.0)

                # lazy per-op gather issue: op o covers global columns
                # [o*OP_COLS, (o+1)*OP_COLS)
                g_tiles = {}

                def ensure_op(o):
                    if o in g_tiles:
                        return g_tiles[o]
                    k = min(OP_COLS, ncols - o * OP_COLS)
                    g = gpool.tile([128, OP_COLS, 128], f16, tag="g", name="g")
                    ni = k * W
                    gi = nc.gpsimd.dma_gather(
                        out_ap=g[:, :k, :], in_ap=m_dram[:],
                        idxs_ap=idx_t[:, o * OP_COLS * 8:o * OP_COLS * 8 + k * 8],
                        num_idxs=ni, num_idxs_reg=ni, elem_size=128,
                        queue_num=o % NQ)
                    add_dep_helper(gi.ins, junction[op_dep[o]].ins, sync=True,
                                   reason="gather after m rows ready")
                    g_tiles[o] = g
                    return g

                # psum accumulation groups are bank-wide (2 KB = 8 windows
                # of [128, 64] f32): start on the bank's first matmul zeroes
                # the whole bank, stop on its last
                for (b0, b1, clo, chi) in blocks:
                    nwb = b1 - b0
                    zwB = wps.tile([128, WBLK * D], f32, tag="zw", name="zw")
                    bank_of = [(cols[col][0] - b0) // 8
                               for col in range(clo, chi)]
                    first_col = {}
                    last_col = {}
                    for i, bk in enumerate(bank_of):
                        first_col.setdefault(bk, i)
                        last_col[bk] = i
                    for i, col in enumerate(range(clo, chi)):
                        wdw, t, j = cols[col]
                        wi = wdw - b0
                        bk = bank_of[i]
                        g = ensure_op(col // OP_COLS)
                        sub = col % OP_COLS
                        P = ppool.tile([128, 128], f16, tag="P")
                        nc.vector.tensor_tensor(
                            out=P[:],
                            in0=nodeid_t[:, col:col + 1].to_broadcast(
                                [128, 128]),
                            in1=iota[:], op=mybir.AluOpType.is_equal)
                        nc.tensor.matmul(
                            out=zwB[:, wi * D:(wi + 1) * D], lhsT=P[:],
                            rhs=g[:, sub, t * D:(t + 1) * D],
                            start=(first_col[bk] == i),
                            stop=(last_col[bk] == i))

                    # ------ phase 3 for this block, GRP windows at a time ----
                    for s0 in range(b0, b1, GRP):
                        gw = min(GRP, b1 - s0)
                        g3 = s0 // GRP
                        cw3 = gw * 128
                        zgrp = zgp.tile([128, GRP * D], f16, tag="zgt")
                        nc.scalar.copy(
                            out=zgrp[:, :gw * D],
                            in_=zwB[:, (s0 - b0) * D:(s0 - b0 + gw) * D])
                        zTg = zTgs[g3 % 2]
                        for kk in range(gw):
                            pst = pstp.tile([D, 128], f16, tag="pst")
                            nc.tensor.transpose(out=pst[:],
                                                in_=zgrp[:, kk * D:(kk + 1) * D],
                                                identity=ident[:])
                            nc.vector.tensor_copy(
                                out=zTg[:D, kk * 128:(kk + 1) * 128], in_=pst[:])
                        ps3 = psb_pool.tile([D, GRP * 128], f32, tag="psb")
                        nc.tensor.matmul(out=ps3[:, :cw3], lhsT=ub1[:],
                                         rhs=zTg[:, :cw3], start=True, stop=True)
                        g1Tg = g1Tgs[g3 % 2]
                        nc.scalar.activation(out=g1Tg[:D, :cw3],
                                             in_=ps3[:, :cw3], func=Relu)
                        h_sb = p3h.tile([128, GRP * D], f32, tag="h_sb")
                        ps4 = pss_pool.tile([128, GRP * D], f32, tag="pss")
                        for kk in range(gw):
                            nc.tensor.matmul(
                                out=ps4[:, kk * D:(kk + 1) * D],
                                lhsT=g1Tg[:, kk * 128:(kk + 1) * 128],
                                rhs=ub2[:], start=True, stop=True)
                        nc.vector.tensor_scalar_max(
                            out=h_sb[:, :gw * D], in0=ps4[:, :gw * D],
                            scalar1=0.0)
                        nc.sync.dma_start(
                            out=h_out[s0 * 128:(s0 + gw) * 128, :]
                                .rearrange("(t p) d -> p t d", p=128),
                            in_=h_sb[:, :gw * D].rearrange(
                                "p (t d) -> p t d", d=D))

    nc.compile()
    return nc


# ------------------------------------------------------------------- kernel
def kernel(**inputs):
    from concourse.bass_utils import run_bass_kernel_spmd

    NPC, NW, NPAD, MT_ROWS = _dims()
    y = np.asarray(inputs["y"], np.float32)
    src = np.asarray(inputs["src"])
    dst = np.asarray(inputs["dst"])
    Ws = {k: np.asarray(inputs[k], np.float32)
          for k in ("W1", "b1", "W2", "b2", "U1", "c1", "U2", "c2")}

    sched, per_core = _host_schedule(src, dst)
    key = (sched["C"], sched["cols"], sched["op_dep"])
    if key not in _COMPILED:
        _COMPILED[key] = _build_program(sched)
    nc = _COMPILED[key]

    yT = np.zeros((D + 1, MT_ROWS), np.float16)
    yT[:D, :N_NODES] = y.T.astype(np.float16)
    yT[D, :] = 1.0
    wb1 = np.concatenate([Ws["W1"], Ws["b1"][None, :]], axis=0).astype(np.float16)
    wb2 = np.concatenate([Ws["W2"], Ws["b2"][None, :]], axis=0).astype(np.float16)
    ub1 = np.concatenate([Ws["U1"], Ws["c1"][None, :]], axis=0).astype(np.float16)
    ub2 = np.concatenate([Ws["U2"], Ws["c2"][None, :]], axis=0).astype(np.float16)
    iota = np.tile(np.arange(128, dtype=np.float16), (128, 1))

    in_maps = []
    for c in range(NC):
        pc = per_core[c]
        in_maps.append({
            "yT": yT, "wb1": wb1, "wb2": wb2, "ub1": ub1, "ub2": ub2,
            "idx": pc["idx"], "nodeid": pc["nodeid"], "iota128": iota,
        })

    res = run_bass_kernel_spmd(nc, in_maps, core_ids=list(range(NC)),
                               trace=bool(int(os.environ.get("KERNEL_TRACE", "0"))))
    kernel.last_results = res
    kernel.last_exec_time_ns = res.exec_time_ns

    h_full = np.zeros((N_NODES, D), np.float32)
    for c in range(NC):
        out = res.results[c]["h_out"]
        perm = per_core[c]["perm"]
        valid = perm >= 0
        h_full[perm[valid]] = out[valid]
    return h_full


# revision 19
# speedup vs baseline: 1.1416x; 1.1416x over previous
"""GNN message-passing aggregator on 8 Trainium2 NeuronCores.

  h = relu(relu(z @ U1 + c1) @ U2 + c2)
  z = segment_sum(relu(relu(y[src] @ W1 + b1) @ W2 + b2), dst)

Strategy (v2):
  * MLP(y[src]) == MLP(y)[src]: compute the pre-MLP once per node (m table),
    then per-edge work collapses to gather m[src] + segment-sum by dst.
  * Edges sharded by dst ownership (6250 nodes/core): segment-sum is
    core-local, no collectives.
  * The m table is fp16 stored as PAIR rows [25024, 128] (node 2r | node
    2r+1, 256 B/row): pair index = src>>1 fits int16, and 256 B satisfies the
    dma_gather elem-size constraint.  Columns are grouped per (dst-window,
    src-parity) so each column's matmul reads the correct 64-elem half.
  * dma_gather descriptor generation runs on the GPSIMD Q7 core pair
    {2q, 2q+1} selected by queue_num.  With num_swdge_queues=4 and ops
    round-robined over queues 0-3 all 8 Q7 cores generate concurrently:
    measured 2.6 ns/idx vs 8.8 ns/idx single-queue.  This is the kernel's
    critical resource (~157k indices/core).
  * Gather ops never span groups; pad slots use idx=-1 which the ucode trims
    at the op tail, so padding costs no descriptor time.  Per-op dependency
    on the m-table chunk covering its max row lets gathers start while
    phase 1 is still writing the table.
  * All matmuls fp16 (fp32 PSUM): phase 1, the one-hot P scatter matmuls
    (built on DVE from host-streamed nodeid vs iota), and the update MLP.
    Phase 3 is interleaved into the window loop so it pipelines into the
    gather stream; final h is fp32.
"""

import os

import numpy as np

# ---------------------------------------------------------------- constants
N_NODES = 50000
D = 64
NC = 8                      # cores
W = 128                     # window size == psum partitions
OP_COLS = 8                 # gather columns per dma_gather op (8*128=1024 idx)
CHUNK = 512                 # dense-MLP T-major matmul chunk (nodes)
GRP = 4                     # windows per phase-3 group
NQ = 4                      # SWDGE queues (ucode max; 4 Q7 core pairs)
GBUFS = 12                  # gather pool depth
WBLK = 16                   # windows per PSUM accumulator block

_COMPILED = {}


def _dims():
    npc = N_NODES // NC
    nw = (npc + W - 1) // W
    npad = nw * W
    mt_rows = ((N_NODES + 127) // 128) * 128
    return npc, nw, npad, mt_rows


# ------------------------------------------------------------ host schedule
def _host_schedule(src, dst):
    """Shard edges by dst ownership, balance nodes into windows, pack columns
    per (window, src-parity) group, cut ops, compute per-op chunk deps.

    Returns (sched, per_core): sched is the shared compile-time schedule
    (identical across cores); per_core holds the input arrays per core.
    """
    NPC, NW, NPAD, MT_ROWS = _dims()
    MT2 = MT_ROWS // 2

    percore_groups = []
    percore_perm = []
    cnt = np.zeros((NC, NW, 2), np.int64)

    for c in range(NC):
        lo_n, hi_n = c * NPC, (c + 1) * NPC
        sel = (dst >= lo_n) & (dst < hi_n)
        s = src[sel].astype(np.int64)
        d = (dst[sel] - lo_n).astype(np.int64)
        deg = np.bincount(d, minlength=NPC)

        # balance nodes into NW windows by degree (greedy, descending)
        order = np.argsort(-deg, kind="stable")
        wload = np.zeros(NW, np.int64)
        wcount = np.zeros(NW, np.int64)
        assign = np.zeros(NPC, np.int64)
        label = np.zeros(NPC, np.int64)
        for n in order:
            wavail = np.flatnonzero(wcount < W)
            wsel = wavail[np.argmin(wload[wavail])]
            assign[n] = wsel
            label[n] = wcount[wsel]
            wcount[wsel] += 1
            wload[wsel] += deg[n]

        ew = assign[d]
        par = s & 1
        groups = {}
        for wdw in range(NW):
            m_w = ew == wdw
            for t in range(2):
                m_t = m_w & (par == t)
                es = (s[m_t] >> 1)                  # pair-row index
                el = label[d[m_t]]
                o = np.argsort(es, kind="stable")   # src-sorted for locality
                groups[(wdw, t)] = (es[o], el[o])
                cnt[c, wdw, t] = (len(es) + W - 1) // W
        percore_groups.append(groups)
        perm = np.full(NPAD, -1, np.int64)
        perm[assign * W + label] = np.arange(NPC) + lo_n
        percore_perm.append(perm)

    C = cnt.max(axis=0)                             # [NW, 2] columns per group
    ncols = int(C.sum())

    # global column order: window BLOCKS of WBLK, layer-major inside a block
    # (layer l = l-th column of each group, ascending src rows) so early
    # gathers only need early m-table chunks
    cols = []                                       # (window, parity, j)
    blocks = []                                     # (w_lo, w_hi, col_lo, col_hi)
    for b0 in range(0, NW, WBLK):
        b1 = min(b0 + WBLK, NW)
        lo = len(cols)
        maxc = int(C[b0:b1].max())
        for j in range(maxc):
            for wdw in range(b0, b1):
                for t in range(2):
                    if j < int(C[wdw, t]):
                        cols.append((wdw, t, j))
        blocks.append((b0, b1, lo, len(cols)))
    assert len(cols) == ncols

    # ops are plain 8-column cuts of the global column list
    nops = (ncols + OP_COLS - 1) // OP_COLS

    # per-op chunk dependency: max pair-row gathered by any core in the op
    # (chunk ch writes pair rows [ch*CHUNK//2, (ch+1)*CHUNK//2))
    op_dep = []
    for o in range(nops):
        mx = 0
        for (wdw, t, j) in cols[o * OP_COLS:(o + 1) * OP_COLS]:
            for c in range(NC):
                es, _ = percore_groups[c][(wdw, t)]
                end = min((j + 1) * W, len(es))
                if end > j * W:
                    mx = max(mx, int(es[j * W:end].max()))
        op_dep.append(min(mx * 2 // CHUNK, (MT_ROWS - 1) // CHUNK))

    per_core = []
    for c in range(NC):
        groups = percore_groups[c]
        idx = np.zeros(ncols * W, np.int16)         # pad rows gather row 0
        nid = np.full(ncols * W, -1.0, np.float16)  # pad slots contribute 0
        for ci, (wdw, t, j) in enumerate(cols):
            es, el = groups[(wdw, t)]
            lo = j * W
            hi = min((j + 1) * W, len(es))
            n = hi - lo
            if n > 0:
                idx[ci * W:ci * W + n] = es[lo:hi].astype(np.int16)
                nid[ci * W:ci * W + n] = el[lo:hi].astype(np.float16)
        # idx wrapped per-op into [16, ni/16] replicated to 128 partitions
        wraps = []
        for o in range(nops):
            op = idx[o * OP_COLS * W:(o + 1) * OP_COLS * W]
            blk = op.reshape(len(op) // 16, 16).T        # [16, ni/16]
            wraps.append(np.tile(blk, (8, 1)))           # [128, ni/16]
        idx_2d = np.concatenate(wraps, axis=1)
        nodeid_2d = nid.reshape(ncols, W).T
        per_core.append({
            "idx": np.ascontiguousarray(idx_2d),
            "nodeid": np.ascontiguousarray(nodeid_2d),
            "perm": percore_perm[c],
        })

    sched = {"C": tuple(tuple(int(x) for x in row) for row in C),
             "cols": tuple(cols), "blocks": tuple(blocks),
             "op_dep": tuple(op_dep), "ncols": ncols}
    return sched, per_core


# ------------------------------------------------------------- bass program
def _build_program(sched):
    import concourse.bacc as bacc
    import concourse.mybir as mybir
    import concourse.tile as tile
    from concourse.masks import make_identity
    from concourse.tile import add_dep_helper

    f32 = mybir.dt.float32
    f16 = mybir.dt.float16
    i16 = mybir.dt.int16
    Relu = mybir.ActivationFunctionType.Relu

    NPC, NW, NPAD, MT_ROWS = _dims()
    MT2 = MT_ROWS // 2
    C = sched["C"]
    cols = sched["cols"]
    blocks = sched["blocks"]
    op_dep = sched["op_dep"]
    ncols = sched["ncols"]
    NCH = (MT_ROWS + CHUNK - 1) // CHUNK
    NGRP = (NW + GRP - 1) // GRP

    nc = bacc.Bacc(num_swdge_queues=NQ)
    yT_in = nc.dram_tensor("yT", [D + 1, MT_ROWS], f16, kind="ExternalInput")
    wb1_in = nc.dram_tensor("wb1", [D + 1, D], f16, kind="ExternalInput")
    wb2_in = nc.dram_tensor("wb2", [D + 1, D], f16, kind="ExternalInput")
    ub1_in = nc.dram_tensor("ub1", [D + 1, D], f16, kind="ExternalInput")
    ub2_in = nc.dram_tensor("ub2", [D + 1, D], f16, kind="ExternalInput")
    idx_in = nc.dram_tensor("idx", [128, ncols * 8], i16, kind="ExternalInput")
    nodeid_in = nc.dram_tensor("nodeid", [128, ncols], f16, kind="ExternalInput")
    iota_in = nc.dram_tensor("iota128", [128, 128], f16, kind="ExternalInput")
    m_dram = nc.dram_tensor("m_scratch", [MT2, 128], f16, kind="Internal")
    h_out = nc.dram_tensor("h_out", [NPAD, D], f32, kind="ExternalOutput")

    with tile.TileContext(nc) as tc:
        with tc.tile_pool(name="const", bufs=1) as cpool, \
             tc.tile_pool(name="idxp", bufs=1) as idxp, \
             tc.tile_pool(name="psb", bufs=2, space="PSUM") as psb_pool, \
             tc.tile_pool(name="pss", bufs=1, space="PSUM") as pss_pool, \
             tc.tile_pool(name="wps", bufs=2, space="PSUM") as wps, \
             tc.tile_pool(name="pstp", bufs=1, space="PSUM") as pstp:
            wb1 = cpool.tile([D + 1, D], f16, tag="wb1")
            wb2 = cpool.tile([D + 1, D], f16, tag="wb2")
            ub1 = cpool.tile([D + 1, D], f16, tag="ub1")
            ub2 = cpool.tile([D + 1, D], f16, tag="ub2")
            iota = cpool.tile([128, 128], f16, tag="iota")
            ident = cpool.tile([128, 128], f16, tag="ident")
            nc.sync.dma_start(out=wb1[:], in_=wb1_in[:])
            nc.sync.dma_start(out=wb2[:], in_=wb2_in[:])
            nc.sync.dma_start(out=ub1[:], in_=ub1_in[:])
            nc.sync.dma_start(out=ub2[:], in_=ub2_in[:])
            nc.sync.dma_start(out=iota[:], in_=iota_in[:])
            make_identity(nc, ident[:])

            idx_t = idxp.tile([128, ncols * 8], i16, tag="idx")
            nc.sync.dma_start(out=idx_t[:], in_=idx_in[:])
            nodeid_t = idxp.tile([128, ncols], f16, tag="nid")
            nc.sync.dma_start(out=nodeid_t[:], in_=nodeid_in[:])

            # ------------ phase 1: m = relu(relu(y@W1+b1)@W2+b2) -> m_dram ---
            m_writes = []
            with tc.tile_pool(name="p1y", bufs=3) as p1y, \
                 tc.tile_pool(name="p1h", bufs=1) as p1h, \
                 tc.tile_pool(name="p1m", bufs=3) as p1m:
                h1a = p1h.tile([D + 1, CHUNK], f16, tag="h1a")
                h1b = p1h.tile([D + 1, CHUNK], f16, tag="h1b")
                nc.gpsimd.memset(h1a[D:D + 1, :], 1.0)
                nc.gpsimd.memset(h1b[D:D + 1, :], 1.0)
                h1bufs = [h1a, h1b]
                mch = None
                pair_c0, pair_cw = 0, 0
                for ch in range(NCH):
                    c0 = ch * CHUNK
                    cw = min(CHUNK, MT_ROWS - c0)
                    ytile = p1y.tile([D + 1, CHUNK], f16, tag="ytile")
                    nc.sync.dma_start(out=ytile[:, :cw], in_=yT_in[:, c0:c0 + cw])
                    ps = psb_pool.tile([D, CHUNK], f32, tag="psb")
                    nc.tensor.matmul(out=ps[:, :cw], lhsT=wb1[:], rhs=ytile[:, :cw],
                                     start=True, stop=True)
                    h1c = h1bufs[ch % 2]
                    nc.scalar.activation(out=h1c[:D, :cw], in_=ps[:, :cw], func=Relu)
                    if ch % 2 == 0:
                        mch = p1m.tile([128, 2 * (CHUNK // 128) * D], f16,
                                       tag="mch")
                        pair_c0, pair_cw = c0, 0
                    nb = cw // 128
                    half = (ch % 2) * (CHUNK // 128) * D
                    ps2 = pss_pool.tile([128, (CHUNK // 128) * D], f32, tag="pss")
                    for i in range(nb):
                        nc.tensor.matmul(out=ps2[:, i * D:(i + 1) * D],
                                         lhsT=h1c[:, i * 128:(i + 1) * 128],
                                         rhs=wb2[:], start=True, stop=True)
                    nc.vector.tensor_scalar_max(
                        out=mch[:, half:half + nb * D], in0=ps2[:, :nb * D],
                        scalar1=0.0)
                    pair_cw += cw
                    if ch % 2 == 1 or ch == NCH - 1:
                        # write the fp16 pair-row table: node (t*128+p) ->
                        # pair row t*64+p//2, half p%2
                        wri = nc.scalar.dma_start(
                            out=m_dram[pair_c0 // 2:(pair_c0 + pair_cw) // 2, :]
                                .rearrange("(t ph) (pl d) -> (ph pl) t d",
                                           ph=64, pl=2),
                            in_=mch[:, :(pair_cw // 128) * D].rearrange(
                                "p (t d) -> p t d", d=D))
                        m_writes.append(wri)

            # gathers dep directly on the pair-write covering their max row:
            # writes share one HWDGE queue and each spans all 16 DMA engines,
            # so write K completing implies writes 0..K-1 completed
            junction = [m_writes[min(ch // 2, len(m_writes) - 1)]
                        for ch in range(NCH)]

            # ------------ phase 2 + 3 interleaved ---------------------------
            with tc.tile_pool(name="gpool", bufs=GBUFS) as gpool, \
                 tc.tile_pool(name="ppool", bufs=6) as ppool, \
                 tc.tile_pool(name="zg", bufs=3) as zgp, \
                 tc.tile_pool(name="p3z", bufs=1) as p3z, \
                 tc.tile_pool(name="p3g", bufs=1) as p3g, \
                 tc.tile_pool(name="p3h", bufs=2) as p3h:

                # phase-3 staging buffers with the ones-row preset ONCE, so
                # no per-group memset lands in the Pool queue between gathers
                zTgs = [p3z.tile([D + 1, GRP * 128], f16, tag=f"zTg{i}",
                                 name=f"zTg{i}") for i in range(2)]
                g1Tgs = [p3g.tile([D + 1, GRP * 128], f16, tag=f"g1Tg{i}",
                                  name=f"g1Tg{i}") for i in range(2)]
                for tl in zTgs + g1Tgs:
                    nc.vector.memset(tl[D:D + 1, :], 1.0)

                # lazy per-op gather issue: op o covers global columns
                # [o*OP_COLS, (o+1)*OP_COLS)
                g_tiles = {}

                def ensure_op(o):
                    if o in g_tiles:
                        return g_tiles[o]
                    k = min(OP_COLS, ncols - o * OP_COLS)
                    g = gpool.tile([128, OP_COLS, 128], f16, tag="g", name="g")
                    ni = k * W
                    # slice the table to the rows this op can touch: Tile
                    # region-tracks DRAM RAW deps, so a full-table in_ap would
                    # serialize every gather behind ALL of phase 1
                    rows = min((op_dep[o] + 1) * (CHUNK // 2), MT2)
                    gi = nc.gpsimd.dma_gather(
                        out_ap=g[:, :k, :], in_ap=m_dram[0:rows, :],
                        idxs_ap=idx_t[:, o * OP_COLS * 8:o * OP_COLS * 8 + k * 8],
                        num_idxs=ni, num_idxs_reg=ni, elem_size=128,
                        queue_num=o % NQ)
                    add_dep_helper(gi.ins, junction[op_dep[o]].ins, sync=True,
                                   reason="gather after m rows ready")
                    g_tiles[o] = g
                    return g

                # psum accumulation groups are bank-wide (2 KB = 8 windows
                # of [128, 64] f32): start on the bank's first matmul zeroes
                # the whole bank, stop on its last
                for (b0, b1, clo, chi) in blocks:
                    nwb = b1 - b0
                    zwB = wps.tile([128, WBLK * D], f32, tag="zw", name="zw")
                    bank_of = [(cols[col][0] - b0) // 8
                               for col in range(clo, chi)]
                    first_col = {}
                    last_col = {}
                    for i, bk in enumerate(bank_of):
                        first_col.setdefault(bk, i)
                        last_col[bk] = i
                    for i, col in enumerate(range(clo, chi)):
                        wdw, t, j = cols[col]
                        wi = wdw - b0
                        bk = bank_of[i]
                        g = ensure_op(col // OP_COLS)
                        sub = col % OP_COLS
                        P = ppool.tile([128, 128], f16, tag="P")
                        nc.vector.tensor_tensor(
                            out=P[:],
                            in0=nodeid_t[:, col:col + 1].to_broadcast(
                                [128, 128]),
                            in1=iota[:], op=mybir.AluOpType.is_equal)
                        nc.tensor.matmul(
                            out=zwB[:, wi * D:(wi + 1) * D], lhsT=P[:],
                            rhs=g[:, sub, t * D:(t + 1) * D],
                            start=(first_col[bk] == i),
                            stop=(last_col[bk] == i))

                    # ------ phase 3 for this block, GRP windows at a time ----
                    for s0 in range(b0, b1, GRP):
                        gw = min(GRP, b1 - s0)
                        g3 = s0 // GRP
                        cw3 = gw * 128
                        zgrp = zgp.tile([128, GRP * D], f16, tag="zgt")
                        nc.scalar.copy(
                            out=zgrp[:, :gw * D],
                            in_=zwB[:, (s0 - b0) * D:(s0 - b0 + gw) * D])
                        zTg = zTgs[g3 % 2]
                        for kk in range(gw):
                            pst = pstp.tile([D, 128], f16, tag="pst")
                            nc.tensor.transpose(out=pst[:],
                                                in_=zgrp[:, kk * D:(kk + 1) * D],
                                                identity=ident[:])
                            nc.vector.tensor_copy(
                                out=zTg[:D, kk * 128:(kk + 1) * 128], in_=pst[:])
                        ps3 = psb_pool.tile([D, GRP * 128], f32, tag="psb")
                        nc.tensor.matmul(out=ps3[:, :cw3], lhsT=ub1[:],
                                         rhs=zTg[:, :cw3], start=True, stop=True)
                        g1Tg = g1Tgs[g3 % 2]
                        nc.scalar.activation(out=g1Tg[:D, :cw3],
                                             in_=ps3[:, :cw3], func=Relu)
                        h_sb = p3h.tile([128, GRP * D], f32, tag="h_sb")
                        ps4 = pss_pool.tile([128, GRP * D], f32, tag="pss")
                        for kk in range(gw):
                            nc.tensor.matmul(
                                out=ps4[:, kk * D:(kk + 1) * D],
                                lhsT=g1Tg[:, kk * 128:(kk + 1) * 128],
                                rhs=ub2[:], start=True, stop=True)
                        nc.vector.tensor_scalar_max(
                            out=h_sb[:, :gw * D], in0=ps4[:, :gw * D],
                            scalar1=0.0)
                        nc.sync.dma_start(
                            out=h_out[s0 * 128:(s0 + gw) * 128, :]
                                .rearrange("(t p) d -> p t d", p=128),
                            in_=h_sb[:, :gw * D].rearrange(
                                "p (t d) -> p t d", d=D))

    nc.compile()
    return nc


# ------------------------------------------------------------------- kernel
def kernel(**inputs):
    from concourse.bass_utils import run_bass_kernel_spmd

    NPC, NW, NPAD, MT_ROWS = _dims()
    y = np.asarray(inputs["y"], np.float32)
    src = np.asarray(inputs["src"])
    dst = np.asarray(inputs["dst"])
    Ws = {k: np.asarray(inputs[k], np.float32)
          for k in ("W1", "b1", "W2", "b2", "U1", "c1", "U2", "c2")}

    sched, per_core = _host_schedule(src, dst)
    key = (sched["C"], sched["cols"], sched["op_dep"])
    if key not in _COMPILED:
        _COMPILED[key] = _build_program(sched)
    nc = _COMPILED[key]

    yT = np.zeros((D + 1, MT_ROWS), np.float16)
    yT[:D, :N_NODES] = y.T.astype(np.float16)
    yT[D, :] = 1.0
    wb1 = np.concatenate([Ws["W1"], Ws["b1"][None, :]], axis=0).astype(np.float16)
    wb2 = np.concatenate([Ws["W2"], Ws["b2"][None, :]], axis=0).astype(np.float16)
    ub1 = np.concatenate([Ws["U1"], Ws["c1"][None, :]], axis=0).astype(np.float16)
    ub2 = np.concatenate([Ws["U2"], Ws["c2"][None, :]], axis=0).astype(np.float16)
    iota = np.tile(np.arange(128, dtype=np.float16), (128, 1))

    in_maps = []
    for c in range(NC):
        pc = per_core[c]
        in_maps.append({
            "yT": yT, "wb1": wb1, "wb2": wb2, "ub1": ub1, "ub2": ub2,
            "idx": pc["idx"], "nodeid": pc["nodeid"], "iota128": iota,
        })

    res = run_bass_kernel_spmd(nc, in_maps, core_ids=list(range(NC)),
                               trace=bool(int(os.environ.get("KERNEL_TRACE", "0"))))
    kernel.last_results = res
    kernel.last_exec_time_ns = res.exec_time_ns

    h_full = np.zeros((N_NODES, D), np.float32)
    for c in range(NC):
        out = res.results[c]["h_out"]
        perm = per_core[c]["perm"]
        valid = perm >= 0
        h_full[perm[valid]] = out[valid]
    return h_full


# revision 21
# speedup vs baseline: 1.3239x; 1.1597x over previous
"""GNN message-passing aggregator on 8 Trainium2 NeuronCores.

  h = relu(relu(z @ U1 + c1) @ U2 + c2)
  z = segment_sum(relu(relu(y[src] @ W1 + b1) @ W2 + b2), dst)

Strategy (v2):
  * MLP(y[src]) == MLP(y)[src]: compute the pre-MLP once per node (m table),
    then per-edge work collapses to gather m[src] + segment-sum by dst.
  * Edges sharded by dst ownership (6250 nodes/core): segment-sum is
    core-local, no collectives.
  * The m table is fp16 stored as PAIR rows [25024, 128] (node 2r | node
    2r+1, 256 B/row): pair index = src>>1 fits int16, and 256 B satisfies the
    dma_gather elem-size constraint.  Columns are grouped per (dst-window,
    src-parity) so each column's matmul reads the correct 64-elem half.
  * dma_gather descriptor generation runs on the GPSIMD Q7 core pair
    {2q, 2q+1} selected by queue_num.  With num_swdge_queues=4 and ops
    round-robined over queues 0-3 all 8 Q7 cores generate concurrently:
    measured 2.6 ns/idx vs 8.8 ns/idx single-queue.  This is the kernel's
    critical resource (~157k indices/core).
  * Gather ops are plain 8-column cuts of the window-major column list; pad
    slots gather row 0 with nodeid=-1 so their one-hot P column is zero and
    they contribute nothing.  Gathers depend on the m-table pair-write
    covering their max row (writes share one HWDGE queue, so write K done
    implies writes 0..K-1 done).
  * All matmuls fp16 (fp32 PSUM): phase 1, the one-hot P scatter matmuls
    (built on DVE from host-streamed nodeid vs iota), and the update MLP.
    Phase 3 is interleaved into the window loop so it pipelines into the
    gather stream; final h is fp32.
"""

import os

import numpy as np

# ---------------------------------------------------------------- constants
N_NODES = 50000
D = 64
NC = 8                      # cores
W = 128                     # window size == psum partitions
OP_COLS = 8                 # gather columns per dma_gather op (8*128=1024 idx)
CHUNK = 512                 # dense-MLP T-major matmul chunk (nodes)
GRP = 4                     # windows per phase-3 group
NQ = 4                      # SWDGE queues (ucode max; 4 Q7 core pairs)
GBUFS = 12                  # gather pool depth

_COMPILED = {}


def _dims():
    npc = N_NODES // NC
    nw = (npc + W - 1) // W
    npad = nw * W
    mt_rows = ((N_NODES + 127) // 128) * 128
    return npc, nw, npad, mt_rows


# ------------------------------------------------------------ host schedule
def _host_schedule(src, dst):
    """Shard edges by dst ownership, balance nodes into windows, pack columns
    per (window, src-parity) group, cut ops, compute per-op chunk deps.

    Returns (sched, per_core): sched is the shared compile-time schedule
    (identical across cores); per_core holds the input arrays per core.
    """
    NPC, NW, NPAD, MT_ROWS = _dims()
    MT2 = MT_ROWS // 2

    percore_groups = []
    percore_perm = []
    cnt = np.zeros((NC, NW, 2), np.int64)

    for c in range(NC):
        lo_n, hi_n = c * NPC, (c + 1) * NPC
        sel = (dst >= lo_n) & (dst < hi_n)
        s = src[sel].astype(np.int64)
        d = (dst[sel] - lo_n).astype(np.int64)
        deg = np.bincount(d, minlength=NPC)

        # balance nodes into NW windows by degree (greedy, descending)
        order = np.argsort(-deg, kind="stable")
        wload = np.zeros(NW, np.int64)
        wcount = np.zeros(NW, np.int64)
        assign = np.zeros(NPC, np.int64)
        label = np.zeros(NPC, np.int64)
        for n in order:
            wavail = np.flatnonzero(wcount < W)
            wsel = wavail[np.argmin(wload[wavail])]
            assign[n] = wsel
            label[n] = wcount[wsel]
            wcount[wsel] += 1
            wload[wsel] += deg[n]

        ew = assign[d]
        par = s & 1
        groups = {}
        for wdw in range(NW):
            m_w = ew == wdw
            for t in range(2):
                m_t = m_w & (par == t)
                es = (s[m_t] >> 1)                  # pair-row index
                el = label[d[m_t]]
                o = np.argsort(es, kind="stable")   # src-sorted for locality
                groups[(wdw, t)] = (es[o], el[o])
                cnt[c, wdw, t] = (len(es) + W - 1) // W
        percore_groups.append(groups)
        perm = np.full(NPAD, -1, np.int64)
        perm[assign * W + label] = np.arange(NPC) + lo_n
        percore_perm.append(perm)

    C = cnt.max(axis=0)                             # [NW, 2] columns per group
    ncols = int(C.sum())

    # global column order: window-major, even-parity group then odd
    cols = []                                       # (window, parity, j)
    gcol0 = {}                                      # (w, t) -> first global col
    for wdw in range(NW):
        for t in range(2):
            gcol0[(wdw, t)] = len(cols)
            for j in range(int(C[wdw, t])):
                cols.append((wdw, t, j))
    assert len(cols) == ncols

    # ops are plain 8-column cuts of the global column list
    nops = (ncols + OP_COLS - 1) // OP_COLS

    # per-op chunk dependency: max pair-row gathered by any core in the op
    # (chunk ch writes pair rows [ch*CHUNK//2, (ch+1)*CHUNK//2))
    op_dep = []
    for o in range(nops):
        mx = 0
        for (wdw, t, j) in cols[o * OP_COLS:(o + 1) * OP_COLS]:
            for c in range(NC):
                es, _ = percore_groups[c][(wdw, t)]
                end = min((j + 1) * W, len(es))
                if end > j * W:
                    mx = max(mx, int(es[j * W:end].max()))
        op_dep.append(min(mx * 2 // CHUNK, (MT_ROWS - 1) // CHUNK))

    per_core = []
    for c in range(NC):
        groups = percore_groups[c]
        idx = np.zeros(ncols * W, np.int16)         # pad rows gather row 0
        nid = np.full(ncols * W, -1.0, np.float16)  # pad slots contribute 0
        for ci, (wdw, t, j) in enumerate(cols):
            es, el = groups[(wdw, t)]
            lo = j * W
            hi = min((j + 1) * W, len(es))
            n = hi - lo
            if n > 0:
                idx[ci * W:ci * W + n] = es[lo:hi].astype(np.int16)
                nid[ci * W:ci * W + n] = el[lo:hi].astype(np.float16)
        # idx wrapped per-op into [16, ni/16] replicated to 128 partitions
        blocks = []
        for o in range(nops):
            op = idx[o * OP_COLS * W:(o + 1) * OP_COLS * W]
            blk = op.reshape(len(op) // 16, 16).T        # [16, ni/16]
            blocks.append(np.tile(blk, (8, 1)))          # [128, ni/16]
        idx_2d = np.concatenate(blocks, axis=1)
        nodeid_2d = nid.reshape(ncols, W).T
        per_core.append({
            "idx": np.ascontiguousarray(idx_2d),
            "nodeid": np.ascontiguousarray(nodeid_2d),
            "perm": percore_perm[c],
        })

    sched = {"C": tuple(tuple(int(x) for x in row) for row in C),
             "op_dep": tuple(op_dep), "ncols": ncols}
    return sched, per_core


# ------------------------------------------------------------- bass program
def _build_program(sched):
    import concourse.bacc as bacc
    import concourse.mybir as mybir
    import concourse.tile as tile
    from concourse.masks import make_identity
    from concourse.tile import add_dep_helper

    f32 = mybir.dt.float32
    f16 = mybir.dt.float16
    i16 = mybir.dt.int16
    Relu = mybir.ActivationFunctionType.Relu

    NPC, NW, NPAD, MT_ROWS = _dims()
    MT2 = MT_ROWS // 2
    C = sched["C"]
    op_dep = sched["op_dep"]
    ncols = sched["ncols"]
    NCH = (MT_ROWS + CHUNK - 1) // CHUNK
    NGRP = (NW + GRP - 1) // GRP

    nc = bacc.Bacc(num_swdge_queues=NQ)
    yT_in = nc.dram_tensor("yT", [D + 1, MT_ROWS], f16, kind="ExternalInput")
    wb1_in = nc.dram_tensor("wb1", [D + 1, D], f16, kind="ExternalInput")
    wb2_in = nc.dram_tensor("wb2", [D + 1, D], f16, kind="ExternalInput")
    ub1_in = nc.dram_tensor("ub1", [D + 1, D], f16, kind="ExternalInput")
    ub2_in = nc.dram_tensor("ub2", [D + 1, D], f16, kind="ExternalInput")
    idx_in = nc.dram_tensor("idx", [128, ncols * 8], i16, kind="ExternalInput")
    nodeid_in = nc.dram_tensor("nodeid", [128, ncols], f16, kind="ExternalInput")
    iota_in = nc.dram_tensor("iota128", [128, 128], f16, kind="ExternalInput")
    m_dram = nc.dram_tensor("m_scratch", [MT2, 128], f16, kind="Internal")
    h_out = nc.dram_tensor("h_out", [NPAD, D], f32, kind="ExternalOutput")

    with tile.TileContext(nc) as tc:
        with tc.tile_pool(name="const", bufs=1) as cpool, \
             tc.tile_pool(name="idxp", bufs=1) as idxp, \
             tc.tile_pool(name="psb", bufs=2, space="PSUM") as psb_pool, \
             tc.tile_pool(name="pss", bufs=2, space="PSUM") as pss_pool, \
             tc.tile_pool(name="wps", bufs=2, space="PSUM") as wps, \
             tc.tile_pool(name="pstp", bufs=1, space="PSUM") as pstp:
            wb1 = cpool.tile([D + 1, D], f16, tag="wb1")
            wb2 = cpool.tile([D + 1, D], f16, tag="wb2")
            ub1 = cpool.tile([D + 1, D], f16, tag="ub1")
            ub2 = cpool.tile([D + 1, D], f16, tag="ub2")
            iota = cpool.tile([128, 128], f16, tag="iota")
            ident = cpool.tile([128, 128], f16, tag="ident")
            nc.sync.dma_start(out=wb1[:], in_=wb1_in[:])
            nc.sync.dma_start(out=wb2[:], in_=wb2_in[:])
            nc.sync.dma_start(out=ub1[:], in_=ub1_in[:])
            nc.sync.dma_start(out=ub2[:], in_=ub2_in[:])
            nc.sync.dma_start(out=iota[:], in_=iota_in[:])
            make_identity(nc, ident[:])

            idx_t = idxp.tile([128, ncols * 8], i16, tag="idx")
            nc.sync.dma_start(out=idx_t[:], in_=idx_in[:])
            nodeid_t = idxp.tile([128, ncols], f16, tag="nid")
            nc.sync.dma_start(out=nodeid_t[:], in_=nodeid_in[:])

            # ------------ phase 1: m = relu(relu(y@W1+b1)@W2+b2) -> m_dram ---
            m_writes = []
            with tc.tile_pool(name="p1y", bufs=3) as p1y, \
                 tc.tile_pool(name="p1h", bufs=1) as p1h, \
                 tc.tile_pool(name="p1m", bufs=3) as p1m:
                h1a = p1h.tile([D + 1, CHUNK], f16, tag="h1a")
                h1b = p1h.tile([D + 1, CHUNK], f16, tag="h1b")
                nc.gpsimd.memset(h1a[D:D + 1, :], 1.0)
                nc.gpsimd.memset(h1b[D:D + 1, :], 1.0)
                h1bufs = [h1a, h1b]
                mch = None
                pair_c0, pair_cw = 0, 0
                for ch in range(NCH):
                    c0 = ch * CHUNK
                    cw = min(CHUNK, MT_ROWS - c0)
                    ytile = p1y.tile([D + 1, CHUNK], f16, tag="ytile")
                    nc.sync.dma_start(out=ytile[:, :cw], in_=yT_in[:, c0:c0 + cw])
                    ps = psb_pool.tile([D, CHUNK], f32, tag="psb")
                    nc.tensor.matmul(out=ps[:, :cw], lhsT=wb1[:], rhs=ytile[:, :cw],
                                     start=True, stop=True)
                    h1c = h1bufs[ch % 2]
                    nc.scalar.activation(out=h1c[:D, :cw], in_=ps[:, :cw], func=Relu)
                    if ch % 2 == 0:
                        mch = p1m.tile([128, 2 * (CHUNK // 128) * D], f16,
                                       tag="mch")
                        pair_c0, pair_cw = c0, 0
                    nb = cw // 128
                    half = (ch % 2) * (CHUNK // 128) * D
                    ps2 = pss_pool.tile([128, (CHUNK // 128) * D], f32, tag="pss")
                    for i in range(nb):
                        nc.tensor.matmul(out=ps2[:, i * D:(i + 1) * D],
                                         lhsT=h1c[:, i * 128:(i + 1) * 128],
                                         rhs=wb2[:], start=True, stop=True)
                    nc.vector.tensor_scalar_max(
                        out=mch[:, half:half + nb * D], in0=ps2[:, :nb * D],
                        scalar1=0.0)
                    pair_cw += cw
                    if ch % 2 == 1 or ch == NCH - 1:
                        # write the fp16 pair-row table: node (t*128+p) ->
                        # pair row t*64+p//2, half p%2
                        wri = nc.scalar.dma_start(
                            out=m_dram[pair_c0 // 2:(pair_c0 + pair_cw) // 2, :]
                                .rearrange("(t ph) (pl d) -> (ph pl) t d",
                                           ph=64, pl=2),
                            in_=mch[:, :(pair_cw // 128) * D].rearrange(
                                "p (t d) -> p t d", d=D))
                        m_writes.append(wri)

            # gathers dep directly on the pair-write covering their max row:
            # writes share one HWDGE queue and each spans all 16 DMA engines,
            # so write K completing implies writes 0..K-1 completed
            junction = [m_writes[min(ch // 2, len(m_writes) - 1)]
                        for ch in range(NCH)]

            # ------------ phase 2 + 3 interleaved ---------------------------
            with tc.tile_pool(name="gpool", bufs=GBUFS) as gpool, \
                 tc.tile_pool(name="ppool", bufs=6) as ppool, \
                 tc.tile_pool(name="zg", bufs=3) as zgp, \
                 tc.tile_pool(name="p3z", bufs=1) as p3z, \
                 tc.tile_pool(name="p3g", bufs=1) as p3g, \
                 tc.tile_pool(name="p3h", bufs=2) as p3h:

                # phase-3 staging buffers with the ones-row preset ONCE, so
                # no per-group memset lands in the Pool queue between gathers
                zTgs = [p3z.tile([D + 1, GRP * 128], f16, tag=f"zTg{i}",
                                 name=f"zTg{i}") for i in range(2)]
                g1Tgs = [p3g.tile([D + 1, GRP * 128], f16, tag=f"g1Tg{i}",
                                  name=f"g1Tg{i}") for i in range(2)]
                for tl in zTgs + g1Tgs:
                    nc.vector.memset(tl[D:D + 1, :], 1.0)

                # lazy per-op gather issue: op o covers global columns
                # [o*OP_COLS, (o+1)*OP_COLS)
                g_tiles = {}

                def ensure_op(o):
                    if o in g_tiles:
                        return g_tiles[o]
                    k = min(OP_COLS, ncols - o * OP_COLS)
                    g = gpool.tile([128, OP_COLS, 128], f16, tag="g", name="g")
                    ni = k * W
                    gi = nc.gpsimd.dma_gather(
                        out_ap=g[:, :k, :], in_ap=m_dram[:],
                        idxs_ap=idx_t[:, o * OP_COLS * 8:o * OP_COLS * 8 + k * 8],
                        num_idxs=ni, num_idxs_reg=ni, elem_size=128,
                        queue_num=o % NQ)
                    add_dep_helper(gi.ins, junction[op_dep[o]].ins, sync=True,
                                   reason="gather after m rows ready")
                    g_tiles[o] = g
                    return g

                gcol = 0
                zgrp = None
                for g3 in range(NGRP):
                    gw = min(GRP, NW - g3 * GRP)
                    zgrp = zgp.tile([128, gw * D], f16, tag="zgt")
                    for wi in range(gw):
                        wdw = g3 * GRP + wi
                        total = C[wdw][0] + C[wdw][1]
                        zw = wps.tile([128, D], f32, tag="zw")
                        ci = 0
                        for t in range(2):
                            for _ in range(C[wdw][t]):
                                col = gcol
                                gcol += 1
                                g = ensure_op(col // OP_COLS)
                                sub = col % OP_COLS
                                P = ppool.tile([128, 128], f16, tag="P")
                                nc.vector.tensor_tensor(
                                    out=P[:],
                                    in0=nodeid_t[:, col:col + 1].to_broadcast(
                                        [128, 128]),
                                    in1=iota[:], op=mybir.AluOpType.is_equal)
                                nc.tensor.matmul(
                                    out=zw[:], lhsT=P[:],
                                    rhs=g[:, sub, t * D:(t + 1) * D],
                                    start=(ci == 0), stop=(ci == total - 1))
                                ci += 1
                        nc.scalar.copy(out=zgrp[:, wi * D:(wi + 1) * D],
                                       in_=zw[:])

                    # ------ phase 3 for this group of windows ----------------
                    cw3 = gw * 128
                    zTg = zTgs[g3 % 2]
                    for kk in range(gw):
                        pst = pstp.tile([D, 128], f16, tag="pst")
                        nc.tensor.transpose(out=pst[:],
                                            in_=zgrp[:, kk * D:(kk + 1) * D],
                                            identity=ident[:])
                        nc.vector.tensor_copy(
                            out=zTg[:D, kk * 128:(kk + 1) * 128], in_=pst[:])
                    ps3 = psb_pool.tile([D, GRP * 128], f32, tag="psb")
                    nc.tensor.matmul(out=ps3[:, :cw3], lhsT=ub1[:],
                                     rhs=zTg[:, :cw3], start=True, stop=True)
                    g1Tg = g1Tgs[g3 % 2]
                    nc.scalar.activation(out=g1Tg[:D, :cw3], in_=ps3[:, :cw3],
                                         func=Relu)
                    h_sb = p3h.tile([128, GRP * D], f32, tag="h_sb")
                    ps4 = pss_pool.tile([128, GRP * D], f32, tag="pss")
                    for kk in range(gw):
                        nc.tensor.matmul(out=ps4[:, kk * D:(kk + 1) * D],
                                         lhsT=g1Tg[:, kk * 128:(kk + 1) * 128],
                                         rhs=ub2[:], start=True, stop=True)
                    nc.vector.tensor_scalar_max(
                        out=h_sb[:, :gw * D], in0=ps4[:, :gw * D], scalar1=0.0)
                    nc.sync.dma_start(
                        out=h_out[g3 * GRP * 128:(g3 * GRP + gw) * 128, :]
                            .rearrange("(t p) d -> p t d", p=128),
                        in_=h_sb[:, :gw * D].rearrange("p (t d) -> p t d", d=D))

    nc.compile()
    return nc


# ------------------------------------------------------------------- kernel
def kernel(**inputs):
    from concourse.bass_utils import run_bass_kernel_spmd

    NPC, NW, NPAD, MT_ROWS = _dims()
    y = np.asarray(inputs["y"], np.float32)
    src = np.asarray(inputs["src"])
    dst = np.asarray(inputs["dst"])
    Ws = {k: np.asarray(inputs[k], np.float32)
          for k in ("W1", "b1", "W2", "b2", "U1", "c1", "U2", "c2")}

    sched, per_core = _host_schedule(src, dst)
    key = (sched["C"], sched["op_dep"])
    if key not in _COMPILED:
        _COMPILED[key] = _build_program(sched)
    nc = _COMPILED[key]

    yT = np.zeros((D + 1, MT_ROWS), np.float16)
    yT[:D, :N_NODES] = y.T.astype(np.float16)
    yT[D, :] = 1.0
    wb1 = np.concatenate([Ws["W1"], Ws["b1"][None, :]], axis=0).astype(np.float16)
    wb2 = np.concatenate([Ws["W2"], Ws["b2"][None, :]], axis=0).astype(np.float16)
    ub1 = np.concatenate([Ws["U1"], Ws["c1"][None, :]], axis=0).astype(np.float16)
    ub2 = np.concatenate([Ws["U2"], Ws["c2"][None, :]], axis=0).astype(np.float16)
    iota = np.tile(np.arange(128, dtype=np.float16), (128, 1))

    in_maps = []
    for c in range(NC):
        pc = per_core[c]
        in_maps.append({
            "yT": yT, "wb1": wb1, "wb2": wb2, "ub1": ub1, "ub2": ub2,
            "idx": pc["idx"], "nodeid": pc["nodeid"], "iota128": iota,
        })

    res = run_bass_kernel_spmd(nc, in_maps, core_ids=list(range(NC)),
                               trace=bool(int(os.environ.get("KERNEL_TRACE", "0"))))
    kernel.last_results = res
    kernel.last_exec_time_ns = res.exec_time_ns

    h_full = np.zeros((N_NODES, D), np.float32)
    for c in range(NC):
        out = res.results[c]["h_out"]
        perm = per_core[c]["perm"]
        valid = perm >= 0
        h_full[perm[valid]] = out[valid]
    return h_full
